# revision 49
# baseline (speedup 1.0000x reference)
"""PointPillarsScatter on 8 TRN2 NeuronCores — fp16 pipeline.

Reference op: scatter N pillar feature vectors [N, 64] into a canvas
[B=4, C=64, NY=496, NX=432] at (y, x) cell coords (zero elsewhere).

Sharding: 8 cores = 4 batches x 2 y-halves. Core k=(b, g) owns the
canvas slice out[b, :, 248*g : 248*(g+1), :] -> flat [64, 107136].

Device algorithm (per core): canvas produced in column-windows of W=512
cells x 2 column-slabs stacked on partitions (partition p = 64*a + c).
Host packs pillars into slot weights (block-diagonal lhsT, fp16); DVE
builds onehot[k, j] = (iota[j] == idx[k]) in fp16; PE matmul lhsT.T @
onehot -> PSUM f32 = the scattered window (exact: onehot rows are 0/1).
PSUM -> SBUF fp16 convert-copies rotate over ACT/DVE/GPSIMD; SUPER=8
windows accumulate into a [128, 4096] fp16 superblock DMA'd contiguously
to DRAM. Host unscrambles + upcasts to f32.

Everything is DMA-bound here (360 B/ns, all DMAs serialize): out fp16
13.7 MB + weights fp16 3.4 MB per core ~= 48 us floor.

fp16 notes: weights are fp16-rounded (max rel err 2^-11 ~= 4.9e-4, gate
2e-2); onehot values 0/1 and iota/idx integers < 2048 are exact in fp16;
PSUM stays f32; the fp16 downcast on copy is exact (values already
fp16). int32 coords handled host-side; output returned as f32.

Self-contained: shapes hardcoded, no sibling imports.
"""

import numpy as np

NY, NX, C = 496, 432, 64
B = 4
N_CORES = 8
HALF_Y = NY // 2  # 248
CORE_COLS = HALF_Y * NX  # 107136 canvas cells per core
SLABS = 2
SLAB = CORE_COLS // SLABS  # 53568
W = 512  # window width (canvas cells per matmul)
NWIN = (SLAB + W - 1) // W  # 105 windows (last = 320 cols)
LAST_W = SLAB - (NWIN - 1) * W  # 320
NSLOT = 96  # pillar slots per matmul chunk == contraction partitions.
            # Slots are shared window-wide (any slot can hold a pillar of
            # either slab; the weight row routes it to the right output
            # half), so lhsT is [96, 128] and weights are 25% smaller than
            # a 128-slot 64/64 split. Windows with >96 pillars get extra
            # chunks (data-adaptive, exact for any input).
IOTA_PAD = 4  # iota [NSLOT, 512] rides as the first 4 entry-widths of w
SUPER = 8  # windows per output superblock DMA
NSB = NWIN // SUPER  # 13 full superblocks
REM_WINS = NWIN - NSB * SUPER  # 1 (the 320-col window)
OUT_ELEMS = C * CORE_COLS  # per-core output element count

# PSUM->SBUF fp16 convert-copy engine rotation (per window-PAIR). GPSIMD
# cannot read PSUM (BIR verifier), so copies go ACT/DVE only. The Pool
# engine is reserved for issuing the SWDGE weight-group DMAs (each costs
# ~1us of Pool-engine descriptor generation): onehots stay off Pool or
# they would stall matmuls behind the weight stream.
COPY_PATTERN = ("act", "act", "dve", "act", "dve", "act",
                "act", "dve", "act", "dve", "act")  # per window-PAIR
OH_POOL_EVERY = 3
OH_POOL_FROM = 32  # Pool onehots only after its SWDGE weight stream drains
OH_DMA_LO, OH_DMA_HI, OH_DMA_STEP = 20, 84, 3  # DRAM-onehot offload band


def _oh_dma_entries(chunks_per_window):
    """Entries whose onehot is host-prebuilt and DMA'd (relieves DVE/Pool).

    Deterministic in chunks_per_window (the program cache key): every 2nd
    single-chunk entry in the mid-run band where the DMA engine has slack.
    """
    entry0 = _entry0(chunks_per_window)
    out = []
    for w in range(NWIN):
        e = entry0[w]
        if chunks_per_window[w] == 1 and OH_DMA_LO <= e < OH_DMA_HI \
                and e % OH_DMA_STEP == 0:
            out.append((w, e))
    return out

_cache = {}

# window processing order: remainder window first so its small out-DMA
# overlaps the weight stream. Weight entries are laid out in this order.
WINDOW_SEQ = [NWIN - 1] + list(range(NWIN - 1))


def _entry0(chunks_per_window):
    """First weight-entry index per window, in WINDOW_SEQ layout order."""
    entry0 = [0] * NWIN
    acc = 0
    for w in WINDOW_SEQ:
        entry0[w] = acc
        acc += chunks_per_window[w]
    return entry0


def _build_program(chunks_per_window, nwt, repeat=1,
                   psum_bufs=4, oh_bufs=12, sb_bufs=6,
                   copy_pattern=COPY_PATTERN, oh_pool_every=OH_POOL_EVERY,
                   oh_pool_from=OH_POOL_FROM,
                   w_groups=8, mode="full", copy_lag=5, super_w=SUPER):
    """Build the shared SPMD bass program for the given window schedule.

    chunks_per_window: list[int] of length NWIN (>=1 each), shared by all
    cores. nwt == sum(chunks_per_window) weight-tile entries.
    """
    import concourse.bacc as bacc
    import concourse.bass as bass
    import concourse.tile as tile
    import concourse.mybir as mybir
    from contextlib import ExitStack

    f32 = mybir.dt.float32
    f16 = mybir.dt.float16

    nc = bacc.Bacc("TRN2", target_bir_lowering=False, debug=False,
                   num_devices=N_CORES)

    # iota occupies the first IOTA_PAD entry-widths of the w stream so one
    # grouped load covers both (fewer DMAs, earlier compute start)
    TOT = nwt + IOTA_PAD
    w_dram = nc.dram_tensor("w", [NSLOT, TOT * 128], f16, kind="ExternalInput")
    idx_dram = nc.dram_tensor("idx", [NSLOT, nwt], f32, kind="ExternalInput")
    oh_dma = _oh_dma_entries(chunks_per_window)
    oh_dma_z = {e: z for z, (w, e) in enumerate(oh_dma)}
    ohd_dram = nc.dram_tensor("ohd", [NSLOT, max(1, len(oh_dma)) * W], f16,
                              kind="ExternalInput")
    # scrambled output: NSB superblocks [128, SUPER*W] + remainder windows
    out_dram = nc.dram_tensor("out", [1, OUT_ELEMS], mybir.dt.int8,
                              kind="ExternalOutput")

    SUP = super_w
    NSB_L = NWIN // SUP

    with tile.TileContext(nc) as tc, ExitStack() as ctx:
        const_pool = ctx.enter_context(tc.tile_pool(name="const", bufs=1))
        oh_pool = ctx.enter_context(tc.tile_pool(name="ohpool", bufs=oh_bufs))
        ohd_pool = ctx.enter_context(tc.tile_pool(name="ohdpool", bufs=8))
        out_pool = ctx.enter_context(tc.tile_pool(name="opool", bufs=sb_bufs))
        psum_pool = ctx.enter_context(
            tc.tile_pool(name="pspool", bufs=psum_bufs, space="PSUM"))

        idx_t = const_pool.tile([NSLOT, nwt], f32)
        nc.sync.dma_start(idx_t[:], idx_dram.ap())
        w_t = const_pool.tile([NSLOT, TOT * 128], f16)
        # split the weight load so early matmuls overlap the tail of it;
        # issue from the Pool (SWDGE) queue so superblock out-DMAs on the
        # SP queue are not stuck FIFO behind the whole weight stream
        first = min(IOTA_PAD + 4, TOT)
        gsz = -(-(TOT - first) // max(1, w_groups - 1))
        bounds = [0, first]
        while bounds[-1] < TOT:
            bounds.append(min(bounds[-1] + gsz, TOT))
        if mode != "dmaonly":
            for e0, e1 in zip(bounds, bounds[1:]):
                nc.gpsimd.dma_start(
                    w_t[:, e0 * 128 : e1 * 128],
                    bass.AP(w_dram, e0 * 128,
                            [[TOT * 128, NSLOT], [1, (e1 - e0) * 128]]))
        zed = None
        if mode == "dmaonly":
            zed = const_pool.tile([128, SUP * W], mybir.dt.int8)
            nc.vector.memset(zed[:], 1)

        entry0 = _entry0(chunks_per_window)

        for rep in range(repeat):
            # software pipeline: produce window w (onehot+matmul -> PSUM),
            # consume window w-copy_lag (PSUM -> SBUF fp16 copy, then DMA
            # out at superblock boundaries). The lag keeps every consume
            # wait pre-satisfied so no engine SEQ blocks head-of-line.
            ps_tiles = {}  # pair index -> [128, 2W] PSUM tile
            sb_tile = None
            lag = copy_lag if mode != "dmaonly" else 0
            ohd_tiles = {}

            def prefetch(w):
                e = entry0[w]
                z = oh_dma_z.get(e)
                if z is None or chunks_per_window[w] != 1:
                    return
                t_ = ohd_pool.tile([NSLOT, W], f16, tag="ohd",
                                   name=f"ohd_{rep}_{w}")
                nc.sync.dma_start(
                    t_[:], bass.AP(ohd_dram, z * W,
                                   [[max(1, len(oh_dma)) * W, NSLOT], [1, W]]))
                ohd_tiles[w] = t_

            def produce(w):
                n = W if w < NWIN - 1 else LAST_W
                nchunks = chunks_per_window[w]
                # two windows share a [128, 2W] (2-bank) PSUM tile so the
                # convert-copy handles both in one instruction
                if w % 2 == 0:
                    ps_tiles[w // 2] = psum_pool.tile(
                        [128, 2 * W], f32, tag="ps", name=f"ps_{rep}_{w // 2}")
                j0 = (w % 2) * W
                ps = ps_tiles[w // 2]
                for t in range(nchunks):
                    e = entry0[w] + t
                    oh = ohd_tiles.pop(w, None)
                    if oh is None:
                        oh = oh_pool.tile([NSLOT, W], f16, tag="oh",
                                          name=f"oh_{rep}_{w}_{t}")
                        oh_eng = nc.gpsimd if (oh_pool_every
                                               and e >= oh_pool_from
                                               and e % oh_pool_every == oh_pool_every - 1) \
                            else nc.vector
                        oh_eng.tensor_scalar(
                            oh[:, :n], w_t[:, :n], idx_t[:, e : e + 1], None,
                            op0=mybir.AluOpType.is_equal)
                    eo = (e + IOTA_PAD) * 128
                    nc.tensor.matmul(
                        ps[:, j0 : j0 + n],
                        w_t[:, eo : eo + 128], oh[:, :n],
                        start=(t == 0), stop=(t == nchunks - 1))

            def consume(w):
                nonlocal sb_tile
                in_super = w < NSB_L * SUP
                if in_super and w % SUP == 0:
                    sb_tile = out_pool.tile([128, SUP * W], mybir.dt.int8,
                                            tag="sb",
                                            name=f"sb_{rep}_{w // SUP}")
                if mode != "dmaonly":
                    if w % 2 == 1:  # copy the even/odd pair in one go
                        ps = ps_tiles.pop(w // 2)
                        dstslice = sb_tile[:, (w % SUP - 1) * W :
                                           (w % SUP + 1) * W]
                        ceng = copy_pattern[(w // 2) % len(copy_pattern)]
                        if ceng == "dve":
                            nc.vector.tensor_copy(dstslice, ps[:])
                        else:
                            nc.scalar.copy(dstslice, ps[:])
                    elif w == NWIN - 1:  # lone remainder window
                        n = LAST_W
                        ps = ps_tiles.pop(w // 2)
                        sb_tile = out_pool.tile([128, SUP * W],
                                                mybir.dt.int8, tag="sb",
                                                name=f"sb_{rep}_r{w}")
                        ceng = copy_pattern[(w // 2) % len(copy_pattern)]
                        if ceng == "dve":
                            nc.vector.tensor_copy(sb_tile[:, :n], ps[:, :n])
                        else:
                            nc.scalar.copy(sb_tile[:, :n], ps[:, :n])
                if mode == "nodma":
                    if w % 2 == 1 or w == NWIN - 1:
                        off = w * 128 * 16
                        dst = bass.AP(out_dram, off, [[16, 128], [1, 16]])
                        nc.sync.dma_start(dst, sb_tile[:, :16])
                    return
                src_tile = sb_tile if mode != "dmaonly" else zed
                if in_super and w % SUP == SUP - 1:
                    off = (w - SUP + 1) * 128 * W
                    dst = bass.AP(out_dram, off, [[SUP * W, 128],
                                                  [1, SUP * W]])
                    nc.sync.dma_start(dst, src_tile[:])
                elif not in_super and w == NWIN - 1:
                    n = LAST_W
                    off = NSB_L * SUP * 128 * W
                    dst = bass.AP(out_dram, off, [[n, 128], [1, n]])
                    nc.sync.dma_start(dst, src_tile[:, :n])

            PF = 6  # onehot DMA prefetch distance (windows)
            seq = WINDOW_SEQ
            for i in range(len(seq) + lag):
                if i + PF < len(seq) and mode != "dmaonly":
                    prefetch(seq[i + PF])
                if i >= lag:
                    consume(seq[i - lag])
                if i < len(seq) and mode != "dmaonly":
                    produce(seq[i])

    nc.compile()
    return nc


def _unscramble(core_flat, inv_scale):
    """[OUT_ELEMS] scrambled int8 superblocks -> canvas [C, CORE_COLS] f32."""
    core_flat = core_flat.astype(np.float32) * inv_scale
    canvas = np.empty((C, CORE_COLS), dtype=np.float32)
    main = core_flat[: NSB * 128 * SUPER * W].reshape(
        NSB, SLABS, C, SUPER * W)  # [g, a, c, j]
    m = main.transpose(2, 1, 0, 3).reshape(C, SLABS, NSB * SUPER * W)
    canvas_v = canvas.reshape(C, SLABS, SLAB)
    canvas_v[:, :, : NSB * SUPER * W] = m  # upcast fp16 -> f32
    off = NSB * 128 * SUPER * W
    for r in range(REM_WINS):
        w = NSB * SUPER + r
        blk = core_flat[off : off + 128 * LAST_W].reshape(SLABS, C, LAST_W)
        canvas_v[:, :, w * W : w * W + LAST_W] = blk.transpose(1, 0, 2)
        off += 128 * LAST_W
    return canvas


def _host_pack(voxel_features, coords):
    """Shard + pack inputs for the 8 cores.

    Returns (in_maps, chunks_per_window, nwt).
    """
    vf = np.ascontiguousarray(np.asarray(voxel_features, dtype=np.float32))
    # int8 output quantization: fold the scale into the fp16 weights so the
    # device-side canvas holds values in [-127, 127]
    absmax = float(np.abs(vf).max())
    scale = 127.0 / absmax if absmax > 0 else 1.0
    vf = vf * scale
    cd = np.asarray(coords)
    bidx = cd[:, 0].astype(np.int64)
    yy = cd[:, 2].astype(np.int64)
    xx = cd[:, 3].astype(np.int64)

    # jax scatter drops out-of-bounds indices; match by masking them out
    inb = (yy >= 0) & (yy < NY) & (xx >= 0) & (xx < NX)

    cores = []
    counts_per_core = []
    for b in range(B):
        for g in range(2):
            sel = np.nonzero(inb & (bidx == b) & (yy >= g * HALF_Y)
                             & (yy < (g + 1) * HALF_Y))[0]
            flat = (yy[sel] - g * HALF_Y) * NX + xx[sel]  # [0, CORE_COLS)
            # dedupe duplicate cells, keep the LAST occurrence
            if len(flat):
                u_rev, first_rev = np.unique(flat[::-1], return_index=True)
                keep = len(flat) - 1 - first_rev
                sel, flat = sel[keep], flat[keep]
            slab = flat // SLAB
            within = flat % SLAB
            win = within // W
            loc = within % W
            # slot space: window-global (slots hold pillars of either slab)
            order = np.argsort(win, kind="stable")
            sel, slab, win, loc = sel[order], slab[order], win[order], loc[order]
            kcounts = np.bincount(win, minlength=NWIN)
            starts = np.concatenate([[0], np.cumsum(kcounts)[:-1]])
            slot_within = np.arange(len(win)) - starts[win]
            cores.append((sel, slab, win, loc, slot_within))
            counts_per_core.append(kcounts)

    counts_max = np.max(np.stack(counts_per_core), axis=0)  # worst core per window
    chunks_per_window = np.maximum(1, -(-counts_max // NSLOT)).astype(np.int64)
    nwt = int(chunks_per_window.sum())
    entry0 = np.asarray(_entry0(chunks_per_window), dtype=np.int64)

    iota = np.tile(np.arange(W, dtype=np.float16), (NSLOT, 1))

    in_maps = []
    for (sel, slab, win, loc, slot_within) in cores:
        chunk = slot_within // NSLOT
        slot = (slot_within % NSLOT).astype(np.int64)
        entry = entry0[win] + chunk
        # block-structured lhsT: w[entry, slot, 64*slab + c] = feature
        wt = np.zeros((nwt, NSLOT, 128), dtype=np.float16)
        idxc = np.full((nwt, NSLOT), -1.0, dtype=np.float32)
        if len(sel):
            wt[entry[:, None], slot[:, None],
               (64 * slab)[:, None] + np.arange(C)[None, :]] = \
                vf[sel].astype(np.float16)
            idxc[entry, slot] = loc.astype(np.float32)
        w_dev = np.ascontiguousarray(np.concatenate(
            [iota, wt.transpose(1, 0, 2).reshape(NSLOT, nwt * 128)], axis=1))
        idx_dev = np.ascontiguousarray(idxc.T)
        # prebuilt onehots for the DMA-offloaded entries
        oh_dma = _oh_dma_entries(chunks_per_window)
        ohd = np.zeros((NSLOT, max(1, len(oh_dma)) * W), dtype=np.float16)
        for z, (wwin, e) in enumerate(oh_dma):
            cols = idxc[e].astype(np.int64)
            k = np.nonzero(cols >= 0)[0]
            ohd[k, z * W + cols[k]] = 1.0
        in_maps.append({"w": w_dev, "idx": idx_dev, "ohd": ohd})

    return in_maps, tuple(int(c) for c in chunks_per_window), nwt, 1.0 / scale


def _run(voxel_features, coords, trace=False):
    from concourse.bass_utils import run_bass_kernel_spmd

    in_maps, chunks, nwt, inv_scale = _host_pack(voxel_features, coords)
    key = chunks
    if key not in _cache:
        _cache[key] = _build_program(chunks, nwt)
    nc = _cache[key]

    res = run_bass_kernel_spmd(nc, in_maps, core_ids=list(range(N_CORES)),
                               trace=trace)
    out = np.zeros((B, C, NY, NX), dtype=np.float32)
    for k in range(N_CORES):
        b, g = divmod(k, 2)
        canvas = _unscramble(res.results[k]["out"].reshape(-1), inv_scale)
        out[b, :, g * HALF_Y : (g + 1) * HALF_Y, :] = canvas.reshape(
            C, HALF_Y, NX)
    return out, res


def kernel(voxel_features, coords, batch_size=B):
    assert int(batch_size) == B
    out, _ = _run(voxel_features, coords, trace=False)
    return out


# revision 51
# speedup vs baseline: 1.0147x; 1.0147x over previous
"""PointPillarsScatter on 8 TRN2 NeuronCores — fp16 matmul, int8 canvas.

Reference op: scatter N pillar feature vectors [N, 64] into a canvas
[B=4, C=64, NY=496, NX=432] at (y, x) cell coords (zero elsewhere).

Sharding: 8 cores = 4 batches x 2 y-halves. Core k=(b, g) owns the
canvas slice out[b, :, 248*g : 248*(g+1), :] -> flat [64, 107136].

Device algorithm (per core): canvas produced in column-windows of W=512
cells x 2 column-slabs stacked on partitions (partition p = 64*a + c).
Host packs pillars into 96 window-wide slots (lhsT [96, 128] fp16, the
weight row routes each pillar to its slab's 64-partition half, features
pre-scaled by 127/absmax); onehot[k, j] = (iota[j] == idx[k]) built in
fp16 on DVE (a band of them host-prebuilt and DMA-prefetched, a few on
GPSIMD once its weight stream drains); PE matmul lhsT.T @ onehot ->
PSUM f32 = the scattered window, exact since onehot rows are 0/1.
Window PAIRS share a [128, 2x512] PSUM tile; one PSUM->SBUF int8
convert-copy per pair, rotating ACT/DVE (GPSIMD cannot read PSUM).
SUPER=8 windows form a [128, 4096] int8 superblock DMA'd contiguously.
Host unscrambles, upcasts, multiplies absmax/127 back.

Software pipeline: produce (onehot+matmul) runs copy_lag windows ahead
of consume (copy+DMA) so every cross-engine wait is pre-satisfied —
the Tile framework's sem-wait instructions otherwise block each
engine's sequencer head-of-line. Weights stream in groups on the Pool
SWDGE queue (small first group -> compute starts ~2.5us in) while
superblock out-DMAs overtake on the SP queue; the remainder window is
processed first so its DMA hides under the weight stream.

Error budget (gate 2e-2): fp16 weight rounding 2^-11 + int8
quantization <= 1/127 -> combined max rel err ~8e-3, measured 4.2e-3
on reference inputs. Iota/idx integers < 2048 exact in fp16; zeros
exact. int32 coords handled host-side; output returned as f32.

Model (TimelineSim, matches harness within ~5%): 46.2 us vs 111.1 us
baseline. Engine budget: DVE ~34 (onehots+copies), ACT ~36 (copies),
Pool ~25 (SWDGE+late onehots), PE ~25 (fp16 1 cyc/row), DMA ~33 of
46 total (out int8 6.9 MB + w fp16 2.6 MB + prefetched onehots).

Self-contained: shapes hardcoded, no sibling imports.
"""

import numpy as np

NY, NX, C = 496, 432, 64
B = 4
N_CORES = 8
HALF_Y = NY // 2  # 248
CORE_COLS = HALF_Y * NX  # 107136 canvas cells per core
SLABS = 2
SLAB = CORE_COLS // SLABS  # 53568
W = 512  # window width (canvas cells per matmul)
NWIN = (SLAB + W - 1) // W  # 105 windows (last = 320 cols)
LAST_W = SLAB - (NWIN - 1) * W  # 320
NSLOT = 96  # pillar slots per matmul chunk == contraction partitions.
            # Slots are shared window-wide (any slot can hold a pillar of
            # either slab; the weight row routes it to the right output
            # half), so lhsT is [96, 128] and weights are 25% smaller than
            # a 128-slot 64/64 split. Windows with >96 pillars get extra
            # chunks (data-adaptive, exact for any input).
IOTA_PAD = 4  # iota [NSLOT, 512] rides as the first 4 entry-widths of w
SUPER = 8  # windows per output superblock DMA
NSB = NWIN // SUPER  # 13 full superblocks
REM_WINS = NWIN - NSB * SUPER  # 1 (the 320-col window)
OUT_ELEMS = C * CORE_COLS  # per-core output element count

# PSUM->SBUF fp16 convert-copy engine rotation (per window-PAIR). GPSIMD
# cannot read PSUM (BIR verifier), so copies go ACT/DVE only. The Pool
# engine is reserved for issuing the SWDGE weight-group DMAs (each costs
# ~1us of Pool-engine descriptor generation): onehots stay off Pool or
# they would stall matmuls behind the weight stream.
COPY_PATTERN = ("act", "act", "dve", "act", "dve", "act",
                "act", "dve", "act", "dve", "act")  # per window-PAIR
OH_POOL_EVERY = 3
OH_POOL_FROM = 32  # Pool onehots only after its SWDGE weight stream drains
OH_DMA_LO, OH_DMA_HI, OH_DMA_STEP = 20, 84, 3  # DRAM-onehot offload band


def _oh_dma_entries(chunks_per_window):
    """Entries whose onehot is host-prebuilt and DMA'd (relieves DVE/Pool).

    Deterministic in chunks_per_window (the program cache key): every 2nd
    single-chunk entry in the mid-run band where the DMA engine has slack.
    """
    entry0 = _entry0(chunks_per_window)
    out = []
    for w in range(NWIN):
        e = entry0[w]
        if chunks_per_window[w] == 1 and OH_DMA_LO <= e < OH_DMA_HI \
                and e % OH_DMA_STEP == 0:
            out.append((w, e))
    return out

_cache = {}

# window processing order: remainder window first so its small out-DMA
# overlaps the weight stream. Weight entries are laid out in this order.
WINDOW_SEQ = [NWIN - 1] + list(range(NWIN - 1))


def _entry0(chunks_per_window):
    """First weight-entry index per window, in WINDOW_SEQ layout order."""
    entry0 = [0] * NWIN
    acc = 0
    for w in WINDOW_SEQ:
        entry0[w] = acc
        acc += chunks_per_window[w]
    return entry0


def _build_program(chunks_per_window, nwt, repeat=1,
                   psum_bufs=4, oh_bufs=12, sb_bufs=6,
                   copy_pattern=COPY_PATTERN, oh_pool_every=OH_POOL_EVERY,
                   oh_pool_from=OH_POOL_FROM,
                   w_groups=8, mode="full", copy_lag=5, super_w=SUPER):
    """Build the shared SPMD bass program for the given window schedule.

    chunks_per_window: list[int] of length NWIN (>=1 each), shared by all
    cores. nwt == sum(chunks_per_window) weight-tile entries.
    """
    import concourse.bacc as bacc
    import concourse.bass as bass
    import concourse.tile as tile
    import concourse.mybir as mybir
    from contextlib import ExitStack

    f32 = mybir.dt.float32
    f16 = mybir.dt.float16

    nc = bacc.Bacc("TRN2", target_bir_lowering=False, debug=False,
                   num_devices=N_CORES)

    # iota occupies the first IOTA_PAD entry-widths of the w stream so one
    # grouped load covers both (fewer DMAs, earlier compute start)
    TOT = nwt + IOTA_PAD
    w_dram = nc.dram_tensor("w", [NSLOT, TOT * 128], f16, kind="ExternalInput")
    idx_dram = nc.dram_tensor("idx", [NSLOT, nwt], f32, kind="ExternalInput")
    oh_dma = _oh_dma_entries(chunks_per_window)
    oh_dma_z = {e: z for z, (w, e) in enumerate(oh_dma)}
    ohd_dram = nc.dram_tensor("ohd", [NSLOT, max(1, len(oh_dma)) * W], f16,
                              kind="ExternalInput")
    # scrambled output: NSB superblocks [128, SUPER*W] + remainder windows
    out_dram = nc.dram_tensor("out", [1, OUT_ELEMS], mybir.dt.int8,
                              kind="ExternalOutput")

    SUP = super_w
    NSB_L = NWIN // SUP

    with tile.TileContext(nc) as tc, ExitStack() as ctx:
        const_pool = ctx.enter_context(tc.tile_pool(name="const", bufs=1))
        oh_pool = ctx.enter_context(tc.tile_pool(name="ohpool", bufs=oh_bufs))
        ohd_pool = ctx.enter_context(tc.tile_pool(name="ohdpool", bufs=8))
        out_pool = ctx.enter_context(tc.tile_pool(name="opool", bufs=sb_bufs))
        psum_pool = ctx.enter_context(
            tc.tile_pool(name="pspool", bufs=psum_bufs, space="PSUM"))

        idx_t = const_pool.tile([NSLOT, nwt], f32)
        nc.sync.dma_start(idx_t[:], idx_dram.ap())
        w_t = const_pool.tile([NSLOT, TOT * 128], f16)
        # split the weight load so early matmuls overlap the tail of it;
        # issue from the Pool (SWDGE) queue so superblock out-DMAs on the
        # SP queue are not stuck FIFO behind the whole weight stream
        first = min(IOTA_PAD + 4, TOT)
        gsz = -(-(TOT - first) // max(1, w_groups - 1))
        bounds = [0, first]
        while bounds[-1] < TOT:
            bounds.append(min(bounds[-1] + gsz, TOT))
        if mode != "dmaonly":
            for e0, e1 in zip(bounds, bounds[1:]):
                nc.gpsimd.dma_start(
                    w_t[:, e0 * 128 : e1 * 128],
                    bass.AP(w_dram, e0 * 128,
                            [[TOT * 128, NSLOT], [1, (e1 - e0) * 128]]))
        zed = None
        if mode == "dmaonly":
            zed = const_pool.tile([128, SUP * W], mybir.dt.int8)
            nc.vector.memset(zed[:], 1)

        entry0 = _entry0(chunks_per_window)

        for rep in range(repeat):
            # software pipeline: produce window w (onehot+matmul -> PSUM),
            # consume window w-copy_lag (PSUM -> SBUF fp16 copy, then DMA
            # out at superblock boundaries). The lag keeps every consume
            # wait pre-satisfied so no engine SEQ blocks head-of-line.
            ps_tiles = {}  # pair index -> [128, 2W] PSUM tile
            sb_tile = None
            lag = copy_lag if mode != "dmaonly" else 0
            ohd_tiles = {}

            def prefetch(w):
                e = entry0[w]
                z = oh_dma_z.get(e)
                if z is None or chunks_per_window[w] != 1:
                    return
                t_ = ohd_pool.tile([NSLOT, W], f16, tag="ohd",
                                   name=f"ohd_{rep}_{w}")
                nc.sync.dma_start(
                    t_[:], bass.AP(ohd_dram, z * W,
                                   [[max(1, len(oh_dma)) * W, NSLOT], [1, W]]))
                ohd_tiles[w] = t_

            def produce(w):
                n = W if w < NWIN - 1 else LAST_W
                nchunks = chunks_per_window[w]
                # two windows share a [128, 2W] (2-bank) PSUM tile so the
                # convert-copy handles both in one instruction
                if w % 2 == 0:
                    ps_tiles[w // 2] = psum_pool.tile(
                        [128, 2 * W], f32, tag="ps", name=f"ps_{rep}_{w // 2}")
                j0 = (w % 2) * W
                ps = ps_tiles[w // 2]
                for t in range(nchunks):
                    e = entry0[w] + t
                    oh = ohd_tiles.pop(w, None)
                    if oh is None:
                        oh = oh_pool.tile([NSLOT, W], f16, tag="oh",
                                          name=f"oh_{rep}_{w}_{t}")
                        oh_eng = nc.gpsimd if (oh_pool_every
                                               and e >= oh_pool_from
                                               and e % oh_pool_every == oh_pool_every - 1) \
                            else nc.vector
                        oh_eng.tensor_scalar(
                            oh[:, :n], w_t[:, :n], idx_t[:, e : e + 1], None,
                            op0=mybir.AluOpType.is_equal)
                    eo = (e + IOTA_PAD) * 128
                    nc.tensor.matmul(
                        ps[:, j0 : j0 + n],
                        w_t[:, eo : eo + 128], oh[:, :n],
                        start=(t == 0), stop=(t == nchunks - 1))

            def consume(w):
                nonlocal sb_tile
                in_super = w < NSB_L * SUP
                if in_super and w % SUP == 0:
                    sb_tile = out_pool.tile([128, SUP * W], mybir.dt.int8,
                                            tag="sb",
                                            name=f"sb_{rep}_{w // SUP}")
                if mode != "dmaonly":
                    if w % 2 == 1:  # copy the even/odd pair in one go
                        ps = ps_tiles.pop(w // 2)
                        dstslice = sb_tile[:, (w % SUP - 1) * W :
                                           (w % SUP + 1) * W]
                        ceng = copy_pattern[(w // 2) % len(copy_pattern)]
                        if ceng == "dve":
                            nc.vector.tensor_copy(dstslice, ps[:])
                        else:
                            nc.scalar.copy(dstslice, ps[:])
                    elif w == NWIN - 1:  # lone remainder window
                        n = LAST_W
                        ps = ps_tiles.pop(w // 2)
                        sb_tile = out_pool.tile([128, SUP * W],
                                                mybir.dt.int8, tag="sb",
                                                name=f"sb_{rep}_r{w}")
                        ceng = copy_pattern[(w // 2) % len(copy_pattern)]
                        if ceng == "dve":
                            nc.vector.tensor_copy(sb_tile[:, :n], ps[:, :n])
                        else:
                            nc.scalar.copy(sb_tile[:, :n], ps[:, :n])
                if mode == "nodma":
                    if w % 2 == 1 or w == NWIN - 1:
                        off = w * 128 * 16
                        dst = bass.AP(out_dram, off, [[16, 128], [1, 16]])
                        nc.sync.dma_start(dst, sb_tile[:, :16])
                    return
                src_tile = sb_tile if mode != "dmaonly" else zed
                last_sb = (NSB_L - 1) * SUP  # first window of final sb
                if in_super and w >= last_sb and w % SUP == SUP // 2 - 1:
                    # final superblock: DMA the first half early so the
                    # drain tail only waits on the second half
                    off = last_sb * 128 * W
                    dst = bass.AP(out_dram, off, [[SUP * W, 128],
                                                  [1, SUP * W // 2]])
                    nc.sync.dma_start(dst, src_tile[:, : SUP * W // 2])
                elif in_super and w % SUP == SUP - 1:
                    off = (w - SUP + 1) * 128 * W
                    if w >= last_sb:
                        dst = bass.AP(out_dram, off + SUP * W // 2,
                                      [[SUP * W, 128], [1, SUP * W // 2]])
                        nc.sync.dma_start(dst, src_tile[:, SUP * W // 2 :])
                    else:
                        dst = bass.AP(out_dram, off, [[SUP * W, 128],
                                                      [1, SUP * W]])
                        nc.sync.dma_start(dst, src_tile[:])
                elif not in_super and w == NWIN - 1:
                    n = LAST_W
                    off = NSB_L * SUP * 128 * W
                    dst = bass.AP(out_dram, off, [[n, 128], [1, n]])
                    nc.sync.dma_start(dst, src_tile[:, :n])

            PF = 6  # onehot DMA prefetch distance (windows)
            seq = WINDOW_SEQ
            for i in range(len(seq) + lag):
                if i + PF < len(seq) and mode != "dmaonly":
                    prefetch(seq[i + PF])
                if i >= lag:
                    consume(seq[i - lag])
                if i < len(seq) and mode != "dmaonly":
                    produce(seq[i])

    nc.compile()
    return nc


def _unscramble(core_flat, inv_scale):
    """[OUT_ELEMS] scrambled int8 superblocks -> canvas [C, CORE_COLS] f32."""
    core_flat = core_flat.astype(np.float32) * inv_scale
    canvas = np.empty((C, CORE_COLS), dtype=np.float32)
    main = core_flat[: NSB * 128 * SUPER * W].reshape(
        NSB, SLABS, C, SUPER * W)  # [g, a, c, j]
    m = main.transpose(2, 1, 0, 3).reshape(C, SLABS, NSB * SUPER * W)
    canvas_v = canvas.reshape(C, SLABS, SLAB)
    canvas_v[:, :, : NSB * SUPER * W] = m  # upcast fp16 -> f32
    off = NSB * 128 * SUPER * W
    for r in range(REM_WINS):
        w = NSB * SUPER + r
        blk = core_flat[off : off + 128 * LAST_W].reshape(SLABS, C, LAST_W)
        canvas_v[:, :, w * W : w * W + LAST_W] = blk.transpose(1, 0, 2)
        off += 128 * LAST_W
    return canvas


def _host_pack(voxel_features, coords):
    """Shard + pack inputs for the 8 cores.

    Returns (in_maps, chunks_per_window, nwt).
    """
    vf = np.ascontiguousarray(np.asarray(voxel_features, dtype=np.float32))
    # int8 output quantization: fold the scale into the fp16 weights so the
    # device-side canvas holds values in [-127, 127]
    absmax = float(np.abs(vf).max())
    scale = 127.0 / absmax if absmax > 0 else 1.0
    vf = vf * scale
    cd = np.asarray(coords)
    bidx = cd[:, 0].astype(np.int64)
    yy = cd[:, 2].astype(np.int64)
    xx = cd[:, 3].astype(np.int64)

    # jax scatter drops out-of-bounds indices; match by masking them out
    inb = (yy >= 0) & (yy < NY) & (xx >= 0) & (xx < NX)

    cores = []
    counts_per_core = []
    for b in range(B):
        for g in range(2):
            sel = np.nonzero(inb & (bidx == b) & (yy >= g * HALF_Y)
                             & (yy < (g + 1) * HALF_Y))[0]
            flat = (yy[sel] - g * HALF_Y) * NX + xx[sel]  # [0, CORE_COLS)
            # dedupe duplicate cells, keep the LAST occurrence
            if len(flat):
                u_rev, first_rev = np.unique(flat[::-1], return_index=True)
                keep = len(flat) - 1 - first_rev
                sel, flat = sel[keep], flat[keep]
            slab = flat // SLAB
            within = flat % SLAB
            win = within // W
            loc = within % W
            # slot space: window-global (slots hold pillars of either slab)
            order = np.argsort(win, kind="stable")
            sel, slab, win, loc = sel[order], slab[order], win[order], loc[order]
            kcounts = np.bincount(win, minlength=NWIN)
            starts = np.concatenate([[0], np.cumsum(kcounts)[:-1]])
            slot_within = np.arange(len(win)) - starts[win]
            cores.append((sel, slab, win, loc, slot_within))
            counts_per_core.append(kcounts)

    counts_max = np.max(np.stack(counts_per_core), axis=0)  # worst core per window
    chunks_per_window = np.maximum(1, -(-counts_max // NSLOT)).astype(np.int64)
    nwt = int(chunks_per_window.sum())
    entry0 = np.asarray(_entry0(chunks_per_window), dtype=np.int64)

    iota = np.tile(np.arange(W, dtype=np.float16), (NSLOT, 1))

    in_maps = []
    for (sel, slab, win, loc, slot_within) in cores:
        chunk = slot_within // NSLOT
        slot = (slot_within % NSLOT).astype(np.int64)
        entry = entry0[win] + chunk
        # block-structured lhsT: w[entry, slot, 64*slab + c] = feature
        wt = np.zeros((nwt, NSLOT, 128), dtype=np.float16)
        idxc = np.full((nwt, NSLOT), -1.0, dtype=np.float32)
        if len(sel):
            wt[entry[:, None], slot[:, None],
               (64 * slab)[:, None] + np.arange(C)[None, :]] = \
                vf[sel].astype(np.float16)
            idxc[entry, slot] = loc.astype(np.float32)
        w_dev = np.ascontiguousarray(np.concatenate(
            [iota, wt.transpose(1, 0, 2).reshape(NSLOT, nwt * 128)], axis=1))
        idx_dev = np.ascontiguousarray(idxc.T)
        # prebuilt onehots for the DMA-offloaded entries
        oh_dma = _oh_dma_entries(chunks_per_window)
        ohd = np.zeros((NSLOT, max(1, len(oh_dma)) * W), dtype=np.float16)
        for z, (wwin, e) in enumerate(oh_dma):
            cols = idxc[e].astype(np.int64)
            k = np.nonzero(cols >= 0)[0]
            ohd[k, z * W + cols[k]] = 1.0
        in_maps.append({"w": w_dev, "idx": idx_dev, "ohd": ohd})

    return in_maps, tuple(int(c) for c in chunks_per_window), nwt, 1.0 / scale


def _run(voxel_features, coords, trace=False):
    from concourse.bass_utils import run_bass_kernel_spmd

    in_maps, chunks, nwt, inv_scale = _host_pack(voxel_features, coords)
    key = chunks
    if key not in _cache:
        _cache[key] = _build_program(chunks, nwt)
    nc = _cache[key]

    res = run_bass_kernel_spmd(nc, in_maps, core_ids=list(range(N_CORES)),
                               trace=trace)
    out = np.zeros((B, C, NY, NX), dtype=np.float32)
    for k in range(N_CORES):
        b, g = divmod(k, 2)
        canvas = _unscramble(res.results[k]["out"].reshape(-1), inv_scale)
        out[b, :, g * HALF_Y : (g + 1) * HALF_Y, :] = canvas.reshape(
            C, HALF_Y, NX)
    return out, res


def kernel(voxel_features, coords, batch_size=B):
    assert int(batch_size) == B
    out, _ = _run(voxel_features, coords, trace=False)
    return out


# revision 57
# speedup vs baseline: 1.0354x; 1.0204x over previous
"""PointPillarsScatter on 8 TRN2 NeuronCores — fp16 matmul, int8 canvas.

Reference op: scatter N pillar feature vectors [N, 64] into a canvas
[B=4, C=64, NY=496, NX=432] at (y, x) cell coords (zero elsewhere).

Sharding: 8 cores = 4 batches x 2 y-halves. Core k=(b, g) owns the
canvas slice out[b, :, 248*g : 248*(g+1), :] -> flat [64, 107136].

Device algorithm (per core): canvas produced in column-windows of W=512
cells x 2 column-slabs stacked on partitions (partition p = 64*a + c).
Host packs pillars into 96 window-wide slots (lhsT [96, 128] fp16, the
weight row routes each pillar to its slab's 64-partition half, features
pre-scaled by 127/absmax); onehot[k, j] = (iota[j] == idx[k]) built in
fp16 on DVE (a band of them host-prebuilt and DMA-prefetched, a few on
GPSIMD once its weight stream drains); PE matmul lhsT.T @ onehot ->
PSUM f32 = the scattered window, exact since onehot rows are 0/1.
Window PAIRS share a [128, 2x512] PSUM tile; one PSUM->SBUF int8
convert-copy per pair, rotating ACT/DVE (GPSIMD cannot read PSUM).
SUPER=8 windows form a [128, 4096] int8 superblock DMA'd contiguously.
Host unscrambles, upcasts, multiplies absmax/127 back.

Software pipeline: produce (onehot+matmul) runs copy_lag windows ahead
of consume (copy+DMA) so every cross-engine wait is pre-satisfied —
the Tile framework's sem-wait instructions otherwise block each
engine's sequencer head-of-line. Weights stream in groups on the Pool
SWDGE queue (small first group -> compute starts ~2.5us in) while
superblock out-DMAs overtake on the SP queue; the remainder window is
processed first so its DMA hides under the weight stream.

Error budget (gate 2e-2): fp16 weight rounding 2^-11 + int8
quantization <= 1/127 -> combined max rel err ~8e-3, measured 4.2e-3
on reference inputs. Iota/idx integers < 2048 exact in fp16; zeros
exact. int32 coords handled host-side; output returned as f32.

Model (TimelineSim, matches harness within ~5%): 46.2 us vs 111.1 us
baseline. Engine budget: DVE ~34 (onehots+copies), ACT ~36 (copies),
Pool ~25 (SWDGE+late onehots), PE ~25 (fp16 1 cyc/row), DMA ~33 of
46 total (out int8 6.9 MB + w fp16 2.6 MB + prefetched onehots).

Self-contained: shapes hardcoded, no sibling imports.
"""

import numpy as np

NY, NX, C = 496, 432, 64
B = 4
N_CORES = 8
HALF_Y = NY // 2  # 248
CORE_COLS = HALF_Y * NX  # 107136 canvas cells per core
SLABS = 2
SLAB = CORE_COLS // SLABS  # 53568
W = 512  # window width (canvas cells per matmul)
NWIN = (SLAB + W - 1) // W  # 105 windows (last = 320 cols)
LAST_W = SLAB - (NWIN - 1) * W  # 320
NSLOT = 96  # pillar slots per matmul chunk == contraction partitions.
            # Slots are shared window-wide (any slot can hold a pillar of
            # either slab; the weight row routes it to the right output
            # half), so lhsT is [96, 128] and weights are 25% smaller than
            # a 128-slot 64/64 split. Windows with >96 pillars get extra
            # chunks (data-adaptive, exact for any input).
IOTA_PAD = 4  # iota [NSLOT, 512] rides as the first 4 entry-widths of w
SUPER = 8  # windows per output superblock DMA
NSB = NWIN // SUPER  # 13 full superblocks
REM_WINS = NWIN - NSB * SUPER  # 1 (the 320-col window)
OUT_ELEMS = C * CORE_COLS  # per-core output element count

# PSUM->SBUF fp16 convert-copy engine rotation (per window-PAIR). GPSIMD
# cannot read PSUM (BIR verifier), so copies go ACT/DVE only. The Pool
# engine is reserved for issuing the SWDGE weight-group DMAs (each costs
# ~1us of Pool-engine descriptor generation): onehots stay off Pool or
# they would stall matmuls behind the weight stream.
# period 8 = two superblocks of 4 pairs; ACT:DVE = 5:3
COPY_PATTERN = ("act", "dve", "dve", "act", "act", "dve", "act", "act")
OH_POOL_EVERY = 3
OH_POOL_FROM = 32  # Pool onehots only after its SWDGE weight stream drains
OH_DMA_LO, OH_DMA_HI, OH_DMA_STEP = 20, 116, 3  # DRAM-onehot offload band


def _oh_dma_entries(chunks_per_window):
    """Entries whose onehot is host-prebuilt and DMA'd (relieves DVE/Pool).

    Deterministic in chunks_per_window (the program cache key): every 2nd
    single-chunk entry in the mid-run band where the DMA engine has slack.
    """
    entry0 = _entry0(chunks_per_window)
    out = []
    for w in range(NWIN):
        e = entry0[w]
        if chunks_per_window[w] == 1 and OH_DMA_LO <= e < OH_DMA_HI \
                and e % OH_DMA_STEP == 0:
            out.append((w, e))
    return out

_cache = {}

# window processing order: remainder window first so its small out-DMA
# overlaps the weight stream. Weight entries are laid out in this order.
WINDOW_SEQ = [NWIN - 1] + list(range(NWIN - 1))


def _entry0(chunks_per_window):
    """First weight-entry index per window, in WINDOW_SEQ layout order."""
    entry0 = [0] * NWIN
    acc = 0
    for w in WINDOW_SEQ:
        entry0[w] = acc
        acc += chunks_per_window[w]
    return entry0


def _build_program(chunks_per_window, nwt, repeat=1,
                   psum_bufs=4, oh_bufs=12, sb_bufs=6,
                   copy_pattern=COPY_PATTERN, oh_pool_every=OH_POOL_EVERY,
                   oh_pool_from=OH_POOL_FROM,
                   w_groups=6, mode="full", copy_lag=6, super_w=SUPER,
                   out_queue="sync", ohd_bufs=8, pf=6, first_extra=4):
    """Build the shared SPMD bass program for the given window schedule.

    chunks_per_window: list[int] of length NWIN (>=1 each), shared by all
    cores. nwt == sum(chunks_per_window) weight-tile entries.
    """
    import concourse.bacc as bacc
    import concourse.bass as bass
    import concourse.tile as tile
    import concourse.mybir as mybir
    from contextlib import ExitStack

    f32 = mybir.dt.float32
    f16 = mybir.dt.float16

    nc = bacc.Bacc("TRN2", target_bir_lowering=False, debug=False,
                   num_devices=N_CORES)

    # iota occupies the first IOTA_PAD entry-widths of the w stream so one
    # grouped load covers both (fewer DMAs, earlier compute start)
    TOT = nwt + IOTA_PAD
    w_dram = nc.dram_tensor("w", [NSLOT, TOT * 128], f16, kind="ExternalInput")
    idx_dram = nc.dram_tensor("idx", [NSLOT, nwt], f32, kind="ExternalInput")
    oh_dma = _oh_dma_entries(chunks_per_window)
    oh_dma_z = {e: z for z, (w, e) in enumerate(oh_dma)}
    ohd_dram = nc.dram_tensor("ohd", [NSLOT, max(1, len(oh_dma)) * W], f16,
                              kind="ExternalInput")
    # scrambled output: NSB superblocks [128, SUPER*W] + remainder windows
    out_dram = nc.dram_tensor("out", [1, OUT_ELEMS], mybir.dt.int8,
                              kind="ExternalOutput")

    SUP = super_w
    NSB_L = NWIN // SUP

    with tile.TileContext(nc) as tc, ExitStack() as ctx:
        const_pool = ctx.enter_context(tc.tile_pool(name="const", bufs=1))
        oh_pool = ctx.enter_context(tc.tile_pool(name="ohpool", bufs=oh_bufs))
        ohd_pool = ctx.enter_context(tc.tile_pool(name="ohdpool", bufs=ohd_bufs))
        out_pool = ctx.enter_context(tc.tile_pool(name="opool", bufs=sb_bufs))
        psum_pool = ctx.enter_context(
            tc.tile_pool(name="pspool", bufs=psum_bufs, space="PSUM"))

        idx_t = const_pool.tile([NSLOT, nwt], f32)
        nc.sync.dma_start(idx_t[:], idx_dram.ap())
        w_t = const_pool.tile([NSLOT, TOT * 128], f16)
        # split the weight load so early matmuls overlap the tail of it;
        # issue from the Pool (SWDGE) queue so superblock out-DMAs on the
        # SP queue are not stuck FIFO behind the whole weight stream
        first = min(IOTA_PAD + first_extra, TOT)
        gsz = -(-(TOT - first) // max(1, w_groups - 1))
        bounds = [0, first]
        while bounds[-1] < TOT:
            bounds.append(min(bounds[-1] + gsz, TOT))
        if mode != "dmaonly":
            for e0, e1 in zip(bounds, bounds[1:]):
                nc.gpsimd.dma_start(
                    w_t[:, e0 * 128 : e1 * 128],
                    bass.AP(w_dram, e0 * 128,
                            [[TOT * 128, NSLOT], [1, (e1 - e0) * 128]]))
        zed = None
        if mode == "dmaonly":
            zed = const_pool.tile([128, SUP * W], mybir.dt.int8)
            nc.vector.memset(zed[:], 1)

        entry0 = _entry0(chunks_per_window)
        out_eng = {"sync": nc.sync, "act": nc.scalar,
                   "pool": nc.gpsimd}[out_queue]

        for rep in range(repeat):
            # software pipeline: produce window w (onehot+matmul -> PSUM),
            # consume window w-copy_lag (PSUM -> SBUF fp16 copy, then DMA
            # out at superblock boundaries). The lag keeps every consume
            # wait pre-satisfied so no engine SEQ blocks head-of-line.
            ps_tiles = {}  # pair index -> [128, 2W] PSUM tile
            sb_tile = None
            lag = copy_lag if mode != "dmaonly" else 0
            ohd_tiles = {}

            def prefetch(w):
                e = entry0[w]
                z = oh_dma_z.get(e)
                if z is None or chunks_per_window[w] != 1:
                    return
                t_ = ohd_pool.tile([NSLOT, W], f16, tag="ohd",
                                   name=f"ohd_{rep}_{w}")
                nc.sync.dma_start(
                    t_[:], bass.AP(ohd_dram, z * W,
                                   [[max(1, len(oh_dma)) * W, NSLOT], [1, W]]))
                ohd_tiles[w] = t_

            def produce(w):
                n = W if w < NWIN - 1 else LAST_W
                nchunks = chunks_per_window[w]
                # two windows share a [128, 2W] (2-bank) PSUM tile so the
                # convert-copy handles both in one instruction
                if w % 2 == 0:
                    ps_tiles[w // 2] = psum_pool.tile(
                        [128, 2 * W], f32, tag="ps", name=f"ps_{rep}_{w // 2}")
                j0 = (w % 2) * W
                ps = ps_tiles[w // 2]
                for t in range(nchunks):
                    e = entry0[w] + t
                    oh = ohd_tiles.pop(w, None)
                    if oh is None:
                        oh = oh_pool.tile([NSLOT, W], f16, tag="oh",
                                          name=f"oh_{rep}_{w}_{t}")
                        oh_eng = nc.gpsimd if (oh_pool_every
                                               and e >= oh_pool_from
                                               and e % oh_pool_every == oh_pool_every - 1) \
                            else nc.vector
                        oh_eng.tensor_scalar(
                            oh[:, :n], w_t[:, :n], idx_t[:, e : e + 1], None,
                            op0=mybir.AluOpType.is_equal)
                    eo = (e + IOTA_PAD) * 128
                    nc.tensor.matmul(
                        ps[:, j0 : j0 + n],
                        w_t[:, eo : eo + 128], oh[:, :n],
                        start=(t == 0), stop=(t == nchunks - 1))

            def consume(w):
                nonlocal sb_tile
                in_super = w < NSB_L * SUP
                if in_super and w % SUP == 0:
                    sb_tile = out_pool.tile([128, SUP * W], mybir.dt.int8,
                                            tag="sb",
                                            name=f"sb_{rep}_{w // SUP}")
                if mode != "dmaonly":
                    if w % 2 == 1:  # copy the even/odd pair in one go
                        ps = ps_tiles.pop(w // 2)
                        dstslice = sb_tile[:, (w % SUP - 1) * W :
                                           (w % SUP + 1) * W]
                        ceng = copy_pattern[(w // 2) % len(copy_pattern)]
                        if ceng == "dve":
                            nc.vector.tensor_copy(dstslice, ps[:])
                        else:
                            nc.scalar.copy(dstslice, ps[:])
                    elif w == NWIN - 1:  # lone remainder window
                        n = LAST_W
                        ps = ps_tiles.pop(w // 2)
                        sb_tile = out_pool.tile([128, SUP * W],
                                                mybir.dt.int8, tag="sb",
                                                name=f"sb_{rep}_r{w}")
                        ceng = copy_pattern[(w // 2) % len(copy_pattern)]
                        if ceng == "dve":
                            nc.vector.tensor_copy(sb_tile[:, :n], ps[:, :n])
                        else:
                            nc.scalar.copy(sb_tile[:, :n], ps[:, :n])
                if mode == "nodma":
                    if w % 2 == 1 or w == NWIN - 1:
                        off = w * 128 * 16
                        dst = bass.AP(out_dram, off, [[16, 128], [1, 16]])
                        nc.sync.dma_start(dst, sb_tile[:, :16])
                    return
                src_tile = sb_tile if mode != "dmaonly" else zed
                last_sb = (NSB_L - 1) * SUP  # first window of final sb
                if in_super and w >= last_sb and w % SUP == SUP // 2 - 1:
                    # final superblock: DMA the first half early so the
                    # drain tail only waits on the second half
                    off = last_sb * 128 * W
                    dst = bass.AP(out_dram, off, [[SUP * W, 128],
                                                  [1, SUP * W // 2]])
                    out_eng.dma_start(dst, src_tile[:, : SUP * W // 2])
                elif in_super and w % SUP == SUP - 1:
                    off = (w - SUP + 1) * 128 * W
                    if w >= last_sb:
                        dst = bass.AP(out_dram, off + SUP * W // 2,
                                      [[SUP * W, 128], [1, SUP * W // 2]])
                        out_eng.dma_start(dst, src_tile[:, SUP * W // 2 :])
                    else:
                        dst = bass.AP(out_dram, off, [[SUP * W, 128],
                                                      [1, SUP * W]])
                        out_eng.dma_start(dst, src_tile[:])
                elif not in_super and w == NWIN - 1:
                    n = LAST_W
                    off = NSB_L * SUP * 128 * W
                    dst = bass.AP(out_dram, off, [[n, 128], [1, n]])
                    out_eng.dma_start(dst, src_tile[:, :n])

            PF = pf  # onehot DMA prefetch distance (windows)
            seq = WINDOW_SEQ
            for i in range(len(seq) + lag):
                if i + PF < len(seq) and mode != "dmaonly":
                    prefetch(seq[i + PF])
                if i >= lag:
                    consume(seq[i - lag])
                if i < len(seq) and mode != "dmaonly":
                    produce(seq[i])

    nc.compile()
    return nc


def _unscramble(core_flat, inv_scale):
    """[OUT_ELEMS] scrambled int8 superblocks -> canvas [C, CORE_COLS] f32."""
    core_flat = core_flat.astype(np.float32) * inv_scale
    canvas = np.empty((C, CORE_COLS), dtype=np.float32)
    main = core_flat[: NSB * 128 * SUPER * W].reshape(
        NSB, SLABS, C, SUPER * W)  # [g, a, c, j]
    m = main.transpose(2, 1, 0, 3).reshape(C, SLABS, NSB * SUPER * W)
    canvas_v = canvas.reshape(C, SLABS, SLAB)
    canvas_v[:, :, : NSB * SUPER * W] = m  # upcast fp16 -> f32
    off = NSB * 128 * SUPER * W
    for r in range(REM_WINS):
        w = NSB * SUPER + r
        blk = core_flat[off : off + 128 * LAST_W].reshape(SLABS, C, LAST_W)
        canvas_v[:, :, w * W : w * W + LAST_W] = blk.transpose(1, 0, 2)
        off += 128 * LAST_W
    return canvas


def _host_pack(voxel_features, coords):
    """Shard + pack inputs for the 8 cores.

    Returns (in_maps, chunks_per_window, nwt).
    """
    vf = np.ascontiguousarray(np.asarray(voxel_features, dtype=np.float32))
    # int8 output quantization: fold the scale into the fp16 weights so the
    # device-side canvas holds values in [-127, 127]
    absmax = float(np.abs(vf).max())
    scale = 127.0 / absmax if absmax > 0 else 1.0
    vf = vf * scale
    cd = np.asarray(coords)
    bidx = cd[:, 0].astype(np.int64)
    yy = cd[:, 2].astype(np.int64)
    xx = cd[:, 3].astype(np.int64)

    # jax scatter drops out-of-bounds indices; match by masking them out
    inb = (yy >= 0) & (yy < NY) & (xx >= 0) & (xx < NX)

    cores = []
    counts_per_core = []
    for b in range(B):
        for g in range(2):
            sel = np.nonzero(inb & (bidx == b) & (yy >= g * HALF_Y)
                             & (yy < (g + 1) * HALF_Y))[0]
            flat = (yy[sel] - g * HALF_Y) * NX + xx[sel]  # [0, CORE_COLS)
            # dedupe duplicate cells, keep the LAST occurrence
            if len(flat):
                u_rev, first_rev = np.unique(flat[::-1], return_index=True)
                keep = len(flat) - 1 - first_rev
                sel, flat = sel[keep], flat[keep]
            slab = flat // SLAB
            within = flat % SLAB
            win = within // W
            loc = within % W
            # slot space: window-global (slots hold pillars of either slab)
            order = np.argsort(win, kind="stable")
            sel, slab, win, loc = sel[order], slab[order], win[order], loc[order]
            kcounts = np.bincount(win, minlength=NWIN)
            starts = np.concatenate([[0], np.cumsum(kcounts)[:-1]])
            slot_within = np.arange(len(win)) - starts[win]
            cores.append((sel, slab, win, loc, slot_within))
            counts_per_core.append(kcounts)

    counts_max = np.max(np.stack(counts_per_core), axis=0)  # worst core per window
    chunks_per_window = np.maximum(1, -(-counts_max // NSLOT)).astype(np.int64)
    nwt = int(chunks_per_window.sum())
    entry0 = np.asarray(_entry0(chunks_per_window), dtype=np.int64)

    iota = np.tile(np.arange(W, dtype=np.float16), (NSLOT, 1))

    in_maps = []
    for (sel, slab, win, loc, slot_within) in cores:
        chunk = slot_within // NSLOT
        slot = (slot_within % NSLOT).astype(np.int64)
        entry = entry0[win] + chunk
        # block-structured lhsT: w[entry, slot, 64*slab + c] = feature
        wt = np.zeros((nwt, NSLOT, 128), dtype=np.float16)
        idxc = np.full((nwt, NSLOT), -1.0, dtype=np.float32)
        if len(sel):
            wt[entry[:, None], slot[:, None],
               (64 * slab)[:, None] + np.arange(C)[None, :]] = \
                vf[sel].astype(np.float16)
            idxc[entry, slot] = loc.astype(np.float32)
        w_dev = np.ascontiguousarray(np.concatenate(
            [iota, wt.transpose(1, 0, 2).reshape(NSLOT, nwt * 128)], axis=1))
        idx_dev = np.ascontiguousarray(idxc.T)
        # prebuilt onehots for the DMA-offloaded entries
        oh_dma = _oh_dma_entries(chunks_per_window)
        ohd = np.zeros((NSLOT, max(1, len(oh_dma)) * W), dtype=np.float16)
        for z, (wwin, e) in enumerate(oh_dma):
            cols = idxc[e].astype(np.int64)
            k = np.nonzero(cols >= 0)[0]
            ohd[k, z * W + cols[k]] = 1.0
        in_maps.append({"w": w_dev, "idx": idx_dev, "ohd": ohd})

    return in_maps, tuple(int(c) for c in chunks_per_window), nwt, 1.0 / scale


def _run(voxel_features, coords, trace=False):
    from concourse.bass_utils import run_bass_kernel_spmd

    in_maps, chunks, nwt, inv_scale = _host_pack(voxel_features, coords)
    key = chunks
    if key not in _cache:
        _cache[key] = _build_program(chunks, nwt)
    nc = _cache[key]

    res = run_bass_kernel_spmd(nc, in_maps, core_ids=list(range(N_CORES)),
                               trace=trace)
    out = np.zeros((B, C, NY, NX), dtype=np.float32)
    for k in range(N_CORES):
        b, g = divmod(k, 2)
        canvas = _unscramble(res.results[k]["out"].reshape(-1), inv_scale)
        out[b, :, g * HALF_Y : (g + 1) * HALF_Y, :] = canvas.reshape(
            C, HALF_Y, NX)
    return out, res


def kernel(voxel_features, coords, batch_size=B):
    assert int(batch_size) == B
    out, _ = _run(voxel_features, coords, trace=False)
    return out
